# revision 1
# baseline (speedup 1.0000x reference)
"""MultiHeadAttention forward on 8 Trainium2 NeuronCores.

Sharding: core c handles batch (c // 4) and a block of 4 heads
(heads 4*(c%4) .. 4*(c%4)+4), i.e. 256 of the 1024 inner features.
Each core computes its batch's full attention for its heads plus the
partial output projection (rows of W_o for its heads); the host sums
the 4 per-batch partials (the W_o all-reduce) and adds the bias terms.

Device dataflow (all transposed layouts, no on-device transposes):
  host supplies Q[b].T, K[b].T, V[b].T, mask[b,0].T (fp16) per core.
  qT = W_q.T @ Q.T   [feat, q-tok]   (PE, fp16 in / fp32 psum)
  kT = W_k.T @ K.T   [feat, kv-tok]
  v  = V.T.T @ W_v   [kv-tok, feat]  (natural layout for PV lhsT)
  scoresT[kv, q] = kT_h.T @ qT_h  per head, psum fp32
  pT = exp(scoresT / 8) * maskT    (ACT exp reads psum; DVE mask mult)
  outT_aug[65, q] = [v_h | 1].T @ pT  accumulated over kv tiles;
    row 64 is the softmax denominator (ones-column trick).
  attnT = outT[0:64] * (1/denom)   (DVE; denom broadcast via DMA)
  partial = attnT.T @ W_o_shard    [q-tok, d_model] psum -> fp32 out
"""

import os

import numpy as np

B = 2
NQ = 2048
NKV = 2048
DM = 1024
H = 16
DH = 64
N_CORES = 8
CORES_PER_BATCH = N_CORES // B  # 4
HPC = H // CORES_PER_BATCH  # 4 heads per core
FPC = HPC * DH  # 256 features per core

_F16 = np.float16

_cache = {}
last_results = None  # stash of BassKernelResults for test harnesses


def _patch_tile_drain():
    """Split the TileContext tail-drain waits: this walrus build rejects
    Drain instructions carrying more than one sync wait."""
    import concourse.mybir as mybir
    import concourse.tile as tile
    from concourse.vector_clock import ScopedClock

    if getattr(tile.TileContext, "_drain_split_patch", False):
        return

    def _drain_and_barrier(self, tick_clock, wait_clock):
        probe = self.nc.sync.nop(nofuse=True)
        wait_clock.add_sem_waits(
            probe.ins, ScopedClock({None: tick_clock.global_clock})
        )
        si = probe.ins.sync_info
        if si is not None and len(si.on_wait) > 1:
            waits = list(si.on_wait)
            probe.ins.sync_info = mybir.SyncInfo(
                on_wait=waits[:1], on_update=list(si.on_update)
            )
            for w in waits[1:]:
                extra = self.nc.sync.nop(nofuse=True)
                extra.ins.sync_info = mybir.SyncInfo(on_wait=[w], on_update=[])
        self.nc.sync.drain()
        self.nc.all_engine_barrier()
        popped = self.nc._tile_sem_poison_stack.pop()
        assert popped is self._sem_poison
        self.nc.clear_and_free_semaphores(list(self.sems.allocated().values()))
        self.nc.all_engine_barrier()

    tile.TileContext._drain_and_barrier = _drain_and_barrier
    tile.TileContext._drain_split_patch = True


def _split_excess_waits(nc, max_waits=1):
    """This walrus build has very few sync-wait slots per ISA instruction
    (a 2-wait TensorScalarPtr and a 3-wait Drain both fail codegen with
    'Too many sync wait commands').  Hoist all but one wait of every
    instruction into dedicated single-wait NOPs in front of it."""
    import concourse.mybir as mybir

    n = 0
    for f in nc.m.functions:
        for b in f.blocks:
            changed = False
            out = []
            for inst in b.instructions:
                si = inst.sync_info
                if si is not None and si.on_wait and len(si.on_wait) > max_waits:
                    changed = True
                    waits = list(si.on_wait)
                    for w in waits[max_waits:]:
                        n += 1
                        out.append(
                            mybir.InstNoOp(
                                name=f"{inst.name}_xw{n}",
                                sync_info=mybir.SyncInfo(on_wait=[w], on_update=[]),
                                bass_nofuse=True,
                                engine=inst.engine,
                            )
                        )
                    inst.sync_info = mybir.SyncInfo(
                        on_wait=waits[:max_waits], on_update=list(si.on_update)
                    )
                out.append(inst)
            if changed:
                b.instructions = out
    return n


def build_nc(dm=DM, nq=NQ, nkv=NKV, hpc=HPC, dh=DH):
    """Build the per-core Bass program (SPMD: same program, per-core data)."""
    import concourse.bass as bass
    import concourse.mybir as mybir
    import concourse.tile as tile

    _patch_tile_drain()

    f16 = mybir.dt.float16
    f32 = mybir.dt.float32
    fpc = hpc * dh
    KT = dm // 128  # contraction tiles for projections
    FT = fpc // 128  # feature partition-tiles (heads per 128-row tile = 2)
    KV = nkv // 128  # kv-token tiles
    NQB = nq // 512  # q blocks of 512
    NCH = max(nq // 1024, 1)  # exp chunks of 1024 columns
    CHW = min(nq, 1024)  # exp chunk width

    nc = bass.Bass(trn_type="TRN2")

    QT = nc.dram_tensor("QT", [dm, nq], f16, kind="ExternalInput")
    KTi = nc.dram_tensor("KTi", [dm, nkv], f16, kind="ExternalInput")
    VTi = nc.dram_tensor("VTi", [dm, nkv], f16, kind="ExternalInput")
    MT = nc.dram_tensor("MT", [nkv, nq], f16, kind="ExternalInput")
    WQ = nc.dram_tensor("WQ", [dm, fpc], f16, kind="ExternalInput")
    WK = nc.dram_tensor("WK", [dm, fpc], f16, kind="ExternalInput")
    WV = nc.dram_tensor("WV", [dm, fpc], f16, kind="ExternalInput")
    WO = nc.dram_tensor("WO", [fpc, dm], f16, kind="ExternalInput")
    BQ = nc.dram_tensor("BQ", [fpc], f32, kind="ExternalInput")
    BK = nc.dram_tensor("BK", [fpc], f32, kind="ExternalInput")
    OUT = nc.dram_tensor("OUT", [nq, dm], f32, kind="ExternalOutput")

    with tile.TileContext(nc) as tc:
        with (
            tc.tile_pool(name="wpool", bufs=1) as wpool,
            tc.tile_pool(name="mpool", bufs=1) as mpool,
            tc.tile_pool(name="big", bufs=1) as bigpool,
            tc.tile_pool(name="xin", bufs=10) as xin,
            tc.tile_pool(name="pt", bufs=6) as ptpool,
            tc.tile_pool(name="outp", bufs=4) as outpool,
            tc.tile_pool(name="psA", bufs=2, space="PSUM") as psA,
            tc.tile_pool(name="psB", bufs=4, space="PSUM") as psB,
            tc.tile_pool(name="dramp", bufs=4, space="DRAM") as dramp,
        ):
            # ---- constants / weights ----
            wq_sb = wpool.tile([128, KT, fpc], f16)
            nc.sync.dma_start(
                out=wq_sb, in_=WQ[:].rearrange("(kt p) f -> p kt f", p=128)
            )
            wk_sb = wpool.tile([128, KT, fpc], f16)
            nc.sync.dma_start(
                out=wk_sb, in_=WK[:].rearrange("(kt p) f -> p kt f", p=128)
            )
            wv_sb = wpool.tile([128, KT, fpc], f16)
            nc.sync.dma_start(
                out=wv_sb, in_=WV[:].rearrange("(kt p) f -> p kt f", p=128)
            )
            bq_sb = wpool.tile([128, FT], f32)
            nc.sync.dma_start(out=bq_sb, in_=BQ[:].rearrange("(t p) -> p t", p=128))
            bk_sb = wpool.tile([128, FT], f32)
            nc.sync.dma_start(out=bk_sb, in_=BK[:].rearrange("(t p) -> p t", p=128))

            qT_sb = bigpool.tile([128, FT, nq], f16)
            kT_sb = bigpool.tile([128, FT, nkv], f16)
            attnT_sb = bigpool.tile([128, FT, nq], f16)
            v_sb = bigpool.tile([128, KV, hpc, dh + 1], f16)
            nc.vector.memset(v_sb[:, :, :, dh : dh + 1], 1.0)

            # ---- q/k projections: xT = W.T @ X.T  -> [feat, tok] ----
            # kt-outer so PE streams behind the input DMAs; tb-halving keeps
            # live psum count at FT*2 <= 4 (the psB pool size).
            for w_sb, b_sb, x_dram, xT_out, ntok in (
                (wq_sb, bq_sb, QT, qT_sb, nq),
                (wk_sb, bk_sb, KTi, kT_sb, nkv),
            ):
                xts = []
                for kt in range(KT):
                    xt = xin.tile([128, ntok], f16, tag="xin")
                    nc.sync.dma_start(
                        out=xt, in_=x_dram[kt * 128 : (kt + 1) * 128, :]
                    )
                    xts.append(xt)
                ntb = ntok // 512
                for tb0 in range(0, ntb, 2):
                    tbs = list(range(tb0, min(tb0 + 2, ntb)))
                    pss = {}
                    for ft in range(FT):
                        for tb in tbs:
                            pss[ft, tb] = psB.tile([128, 512], f32, tag="psB", name=f"pjps_{ft}_{tb}")
                    for kt in range(KT):
                        for ft in range(FT):
                            for tb in tbs:
                                nc.tensor.matmul(
                                    pss[ft, tb],
                                    w_sb[:, kt, ft * 128 : (ft + 1) * 128],
                                    xts[kt][:, tb * 512 : (tb + 1) * 512],
                                    start=(kt == 0),
                                    stop=(kt == KT - 1),
                                )
                    for ft in range(FT):
                        for tb in tbs:
                            nc.vector.tensor_scalar_add(
                                out=xT_out[:, ft, tb * 512 : (tb + 1) * 512],
                                in0=pss[ft, tb],
                                scalar1=b_sb[:, ft : ft + 1],
                            )

            # ---- v projection: v = V @ W_v -> [kv-tok, feat] (natural) ----
            vts = []
            for kt in range(KT):
                vt = xin.tile([128, nkv], f16, tag="xin")
                nc.sync.dma_start(out=vt, in_=VTi[kt * 128 : (kt + 1) * 128, :])
                vts.append(vt)
            for tt0 in range(0, KV, 4):
                tts = list(range(tt0, min(tt0 + 4, KV)))
                pss = {}
                for tt in tts:
                    pss[tt] = psB.tile([128, fpc], f32, tag="psB", name=f"vps_{tt}")
                for kt in range(KT):
                    for tt in tts:
                        nc.tensor.matmul(
                            pss[tt],
                            vts[kt][:, tt * 128 : (tt + 1) * 128],
                            wv_sb[:, kt, :],
                            start=(kt == 0),
                            stop=(kt == KT - 1),
                        )
                for tt in tts:
                    nc.vector.tensor_copy(
                        out=v_sb[:, tt, :, 0:dh],
                        in_=pss[tt].rearrange("p (h d) -> p h d", d=dh),
                    )

            # mask + W_o loads emitted late so they don't compete with the
            # projection input DMAs for HBM bandwidth / queue slots.
            mt_sb = mpool.tile([128, KV, nq], f16)
            for kv in range(KV):
                nc.sync.dma_start(
                    out=mt_sb[:, kv, :],
                    in_=MT[kv * 128 : (kv + 1) * 128, :],
                )
            wo_sb = wpool.tile([128, FT, dm], f16)
            nc.sync.dma_start(
                out=wo_sb, in_=WO[:].rearrange("(kt p) f -> p kt f", p=128)
            )

            # ---- attention per head ----
            for h in range(hpc):
                ft = h // 2
                ro = (h % 2) * 64
                pv_ps = []
                for qb in range(NQB):
                    pvp = psB.tile([dh + 1, 512], f32, tag="psB")
                    pv_ps.append(pvp)
                for kv in range(KV):
                    pt = ptpool.tile([128, nq], f16, tag="pt")
                    for ch in range(NCH):
                        sc = psA.tile([128, CHW], f32, tag="psA")
                        for half in range(CHW // 512):
                            q0 = ch * CHW + half * 512
                            nc.tensor.matmul(
                                sc[:, half * 512 : (half + 1) * 512],
                                kT_sb[ro : ro + 64, ft, kv * 128 : (kv + 1) * 128],
                                qT_sb[ro : ro + 64, ft, q0 : q0 + 512],
                                start=True,
                                stop=True,
                            )
                        nc.scalar.activation(
                            out=pt[:, ch * CHW : (ch + 1) * CHW],
                            in_=sc,
                            func=mybir.ActivationFunctionType.Exp,
                            scale=float(1.0 / np.sqrt(dh)),
                        )
                    nc.vector.tensor_mul(pt, pt, mt_sb[:, kv, :])
                    for qb in range(NQB):
                        nc.tensor.matmul(
                            pv_ps[qb],
                            v_sb[:, kv, h, :],
                            pt[:, qb * 512 : (qb + 1) * 512],
                            start=(kv == 0),
                            stop=(kv == KV - 1),
                        )
                # free the pv psum slots immediately: copy to SBUF, then
                # normalize from the copy off the critical path.
                pv_sb = outpool.tile(
                    [dh + 1, NQB, 512], f32, tag="pvsb", name=f"pvsb_{h}", bufs=2
                )
                for qb in range(NQB):
                    nc.vector.tensor_copy(out=pv_sb[:, qb, :], in_=pv_ps[qb])
                rec = ptpool.tile([1, nq], f16, tag="pt", name=f"rec_{h}")
                with nc.allow_low_precision(reason="fp16 softmax denominators"):
                    nc.vector.reciprocal(
                        out=rec.rearrange("o (b q) -> o b q", b=NQB),
                        in_=pv_sb[dh : dh + 1, :, :],
                    )
                rd = dramp.tile([nq], f16, tag="rd")
                nc.sync.dma_start(out=rd, in_=rec)
                recb = ptpool.tile([64, nq], f16, tag="pt", name=f"recb_{h}")
                nc.sync.dma_start(
                    out=recb, in_=rd.unsqueeze(0).to_broadcast([64, nq])
                )
                for qb in range(NQB):
                    nc.vector.tensor_mul(
                        attnT_sb[ro : ro + 64, ft, qb * 512 : (qb + 1) * 512],
                        pv_sb[0:dh, qb, :],
                        recb[:, qb * 512 : (qb + 1) * 512],
                    )

            # ---- output projection: partial = attnT.T @ W_o ----
            for tt in range(nq // 128):
                for nb in range(dm // 512):
                    ps = psB.tile([128, 512], f32, tag="psB")
                    for kt2 in range(FT):
                        nc.tensor.matmul(
                            ps,
                            attnT_sb[:, kt2, tt * 128 : (tt + 1) * 128],
                            wo_sb[:, kt2, nb * 512 : (nb + 1) * 512],
                            start=(kt2 == 0),
                            stop=(kt2 == FT - 1),
                        )
                    ob = outpool.tile([128, 512], f32, tag="outp")
                    nc.vector.tensor_copy(out=ob, in_=ps)
                    nc.sync.dma_start(
                        out=OUT[tt * 128 : (tt + 1) * 128, nb * 512 : (nb + 1) * 512],
                        in_=ob,
                    )

    if not int(os.environ.get("KERNEL_NO_WAITSPLIT", "0")):
        _split_excess_waits(nc)
    return nc


def _get_nc():
    if "nc" not in _cache:
        _cache["nc"] = build_nc()
    return _cache["nc"]


def kernel(Q, K, V, mask, W_q, b_q, W_k, b_k, W_v, b_v, W_o, b_o):
    global last_results
    from concourse.bass_utils import run_bass_kernel_spmd

    nc = _get_nc()

    # host-side shard prep (layout massaging only)
    qt = [np.ascontiguousarray(Q[b].T).astype(_F16) for b in range(B)]
    kt = [np.ascontiguousarray(K[b].T).astype(_F16) for b in range(B)]
    vt = [np.ascontiguousarray(V[b].T).astype(_F16) for b in range(B)]
    mt = [np.ascontiguousarray(mask[b, 0].T).astype(_F16) for b in range(B)]
    wq = [
        np.ascontiguousarray(W_q[:, g * FPC : (g + 1) * FPC]).astype(_F16)
        for g in range(CORES_PER_BATCH)
    ]
    wk = [
        np.ascontiguousarray(W_k[:, g * FPC : (g + 1) * FPC]).astype(_F16)
        for g in range(CORES_PER_BATCH)
    ]
    wv = [
        np.ascontiguousarray(W_v[:, g * FPC : (g + 1) * FPC]).astype(_F16)
        for g in range(CORES_PER_BATCH)
    ]
    wo = [
        np.ascontiguousarray(W_o[g * FPC : (g + 1) * FPC, :]).astype(_F16)
        for g in range(CORES_PER_BATCH)
    ]
    bq = [
        np.ascontiguousarray(b_q[g * FPC : (g + 1) * FPC]).astype(np.float32)
        for g in range(CORES_PER_BATCH)
    ]
    bk = [
        np.ascontiguousarray(b_k[g * FPC : (g + 1) * FPC]).astype(np.float32)
        for g in range(CORES_PER_BATCH)
    ]

    in_maps = []
    for c in range(N_CORES):
        b, g = c // CORES_PER_BATCH, c % CORES_PER_BATCH
        in_maps.append(
            {
                "QT": qt[b],
                "KTi": kt[b],
                "VTi": vt[b],
                "MT": mt[b],
                "WQ": wq[g],
                "WK": wk[g],
                "WV": wv[g],
                "WO": wo[g],
                "BQ": bq[g],
                "BK": bk[g],
            }
        )

    trace = bool(int(os.environ.get("KERNEL_TRACE", "0")))
    res = run_bass_kernel_spmd(
        nc, in_maps, core_ids=list(range(N_CORES)), trace=trace
    )
    last_results = res

    out = np.zeros((B, NQ, DM), np.float32)
    for c in range(N_CORES):
        out[c // CORES_PER_BATCH] += res.results[c]["OUT"]
        if "OUT1" in res.results[c]:
            out[c // CORES_PER_BATCH] += res.results[c]["OUT1"]
    # v-bias contributes b_v @ W_o to every row post-softmax; b_o is additive.
    out += (
        np.asarray(b_v, np.float32) @ np.asarray(W_o, np.float32)
        + np.asarray(b_o, np.float32)
    )
    return out

